# revision 9
# baseline (speedup 1.0000x reference)
"""Trainium2 Bass kernel for quantized int8 per-channel Conv2d.

Reference semantics (fp32):
  x_f = (x_int8 - 7) * 0.01                      # per-tensor dequant
  w_f = (w_int8 - zp[cout]) * scale[cout]        # per-channel dequant
  y   = round(conv2d_valid(x_f, w_f) + bias[cout])  -> int32

Algorithm: 1D Winograd F(2,3) along the width axis, direct 3-tap
accumulation along height.  Per 2 output columns the width conv needs 4
matmul points instead of 6 -> PE work drops 1.5x vs direct conv:

  y[h, 2p:2p+2] = A^T [ (G w_row) * (B^T d) ],  d = x[h, 2p:2p+4]
  B^T d = [d0-d2, d1+d2, d2-d1, d1-d3]   (all +-1 -> plain tensor_tensor)
  y0 = m0+m1+m2,  y1 = m1-m2-m3

Matmul operands are fp16: transformed inputs are integers |V| <= 256,
exact in fp16; U = G*(w-zp)*0.01*scale carries 2^-11 relative error.
Host folds the (x-7) per-tensor zeropoint into the bias.

Engine split (v2):
 - PE: j-major matmuls in order (1,2,3,0) so the A^T combines pipeline
   per j and the j=0 point lands last (y0 = s12 + m0 finishes the tile).
 - GPSIMD: "deal" copies: strided int8 x -> dense fp16 planes
   e[t]=x[2t], o, e1, o1.  Dense planes let the DVE transforms run in
   2x packed mode instead of 1x strided.
 - ACT: PSUM drains (+bias on j=1) and a share of the deals.
 - DVE: V transforms (4 tensor_tensor on dense fp16), A^T combines,
   magic-number rounding.
Tail: the last (n,m) fuses the j=0 drain+combine (TT reads PSUM) per
chunk and DMAs each chunk on its own queue.
Sharding: data-parallel over batch N=32 across 8 cores.
"""

import numpy as np

import concourse.bass as bass
import concourse.mybir as mybir
from concourse import bacc
from concourse.tile import TileContext
from concourse.bass_utils import run_bass_kernel_spmd

# Problem shapes (hardcoded per contract)
N, CIN, H, W = 32, 256, 56, 56
COUT, KH, KW = 256, 3, 3
HO, WO = H - KH + 1, W - KW + 1          # 54, 54
NCORES = 8
NPER = N // NCORES                        # images per core
HW = H * W                                # 3136
XPAD = HW + 64
KT = CIN // 128                           # 2 cin tiles
MT = COUT // 128                          # 2 cout tiles
NJ = 4                                    # F(2,3) winograd points
NP = 28                                   # col half-pairs per row (27 used)
TP = H * NP                               # 1568 transform cols per point
CH = 18                                   # output rows per chunk
NCH = 3                                   # chunks per (img, m)
NCOLS = CH * 27                           # 486 matmul free dim
NTOT = NCH * NCOLS                        # 1458
NCOLP = NTOT + 2                          # msb plane pitch, 4B aligned
MAGIC = 12582912.0                        # 1.5 * 2**23 fp32 RNE round trick
JORDER = (1, 2, 3, 0)

GMAT = np.array([
    [1, 0, 0],
    [0.5, 0.5, 0.5],
    [0.5, -0.5, 0.5],
    [0, 0, 1],
], dtype=np.float64)

_CACHE = {}


def _build_program():
    nc = bacc.Bacc("TRN2", target_bir_lowering=False, debug=False,
                   num_devices=NCORES)
    dt = mybir.dt
    f16 = dt.float16
    AF = mybir.ActivationFunctionType
    ALU = mybir.AluOpType

    x_d = nc.dram_tensor("x", [NPER, CIN, H, W], dt.int8, kind="ExternalInput")
    # U layout: [j, cin_part, k, r, m, cout_part]  (j outer -> split DMA)
    u_d = nc.dram_tensor("u", [NJ, 128, KT, KH, MT, 128], f16,
                         kind="ExternalInput")
    b2_d = nc.dram_tensor("bias2", [COUT], dt.float32, kind="ExternalInput")
    out_d = nc.dram_tensor("out", [NPER, COUT, HO, WO], dt.int32,
                           kind="ExternalOutput")

    with TileContext(nc) as tc:
        with (
            tc.tile_pool(name="const", bufs=1) as cpool,
            tc.tile_pool(name="xin", bufs=2) as xpool,
            tc.tile_pool(name="deal", bufs=2) as dpool,
            tc.tile_pool(name="vbuf", bufs=2) as vpool,
            tc.tile_pool(name="psum", bufs=7, space="PSUM") as ppool,
            tc.tile_pool(name="msb", bufs=2) as mpool,
            tc.tile_pool(name="csc", bufs=2) as epool,
            tc.tile_pool(name="yb", bufs=2) as ypool,
            tc.tile_pool(name="outb", bufs=2) as opool,
        ):
            # ---- constants: u per j (j=1 first, matmul order), bias ----
            usb = cpool.tile([128, NJ, KT, KH, MT, 128], f16)
            for j in JORDER:
                nc.sync.dma_start(out=usb[:, j], in_=u_d[j])
            b2 = cpool.tile([128, MT], dt.float32)
            nc.sync.dma_start(out=b2[:, :],
                              in_=b2_d.rearrange("(m p) -> p m", p=128))

            # PE warm-up: substantive matmuls so the HAM activity monitor
            # lifts the clock gate to 8/8 before the first real matmul.
            wupw = cpool.tile([128, 128], f16)
            nc.vector.memset(wupw[:, :], 1.0)
            wupx = cpool.tile([128, 512], f16)
            nc.vector.memset(wupx[:, :], 1.0)
            wups = ppool.tile([128, 512], dt.float32, name="wups", tag="wup",
                              bufs=1)
            for _ in range(8):
                nc.tensor.matmul(wups[:, :], wupw[:, :], wupx[:, :],
                                 start=True, stop=True)

            def keepers(rhs, count):
                # matmuls gated on `rhs` readiness: keep the PE's HAM
                # activity window busy while startup DMA/deals land
                for _ in range(count):
                    nc.tensor.matmul(wups[:, 0:rhs.shape[-1]], wupw[:, :],
                                     rhs, start=True, stop=True)

            def xdma(n, xb):
                for k in range(KT):
                    nc.gpsimd.dma_start(
                        out=xb[:, k, 0:HW],
                        in_=x_d[n, k * 128:(k + 1) * 128].rearrange(
                            "p h w -> p (h w)"))

            # deal plane s (0..3): dp[., t] = x[2t + s], dense fp16
            def deal(eng, xb, dp, k, s):
                src = xb[:, k, s:s + 2 * TP].rearrange(
                    "p (t f) -> p t f", f=2)[:, :, 0]
                if eng is nc.scalar:
                    eng.copy(dp[:, k, s], src)
                else:
                    eng.tensor_copy(dp[:, k, s], src)

            # V transform for one j from dense planes (DVE 2x packed)
            def vtrans(dp, vt, k, j):
                e, o, e1, o1 = (dp[:, k, s] for s in range(4))
                if j == 0:
                    nc.vector.tensor_tensor(vt[:, k, 0], e, e1, ALU.subtract)
                elif j == 1:
                    nc.vector.tensor_tensor(vt[:, k, 1], o, e1, ALU.add)
                elif j == 2:
                    nc.vector.tensor_tensor(vt[:, k, 2], e1, o, ALU.subtract)
                else:
                    nc.vector.tensor_tensor(vt[:, k, 3], o, o1, ALU.subtract)

            # ---- image 0 prologue ----
            # GPSIMD touches no SBUF data: its CAST path is ~4.5 cyc/elem
            # AND it steals the DVE's shared SBUF port.  Deals run on DVE
            # (2x_2P, ~1us) and ACT (~1.6us).
            xb0 = xpool.tile([128, KT, XPAD], dt.int8, name="xb")
            dp0 = dpool.tile([128, KT, 4, TP], f16, name="dp")
            vt0 = vpool.tile([128, KT, NJ, TP], f16, name="vt")
            xdma(0, xb0)
            deal(nc.scalar, xb0, dp0, 1, 1)      # o  k1
            deal(nc.scalar, xb0, dp0, 0, 0)      # e  k0
            deal(nc.scalar, xb0, dp0, 1, 0)      # e  k1
            deal(nc.vector, xb0, dp0, 0, 2)      # e1 k0
            deal(nc.vector, xb0, dp0, 0, 1)      # o  k0
            vtrans(dp0, vt0, 0, 1)
            deal(nc.vector, xb0, dp0, 1, 2)      # e1 k1
            vtrans(dp0, vt0, 1, 1)
            vtrans(dp0, vt0, 0, 2)
            vtrans(dp0, vt0, 1, 2)
            deal(nc.vector, xb0, dp0, 0, 3)      # o1 k0
            deal(nc.vector, xb0, dp0, 1, 3)      # o1 k1
            vtrans(dp0, vt0, 0, 3)
            vtrans(dp0, vt0, 1, 3)
            vtrans(dp0, vt0, 0, 0)
            vtrans(dp0, vt0, 1, 0)
            keepers(vt0[:, 0, 1, 0:512], 5)

            xbs, dps, vts = [xb0], [dp0], [vt0]
            for n in range(NPER):
                vt = vts[n]
                last_img = n == NPER - 1
                if not last_img:
                    xbn = xpool.tile([128, KT, XPAD], dt.int8, name="xb")
                    dpn = dpool.tile([128, KT, 4, TP], f16, name="dp")
                    vtn = vpool.tile([128, KT, NJ, TP], f16, name="vt")
                    xdma(n + 1, xbn)
                    xbs.append(xbn)
                    dps.append(dpn)
                    vts.append(vtn)

                for m in range(MT):
                    last = last_img and m == MT - 1
                    msb = mpool.tile([128, NJ, NCOLP], f16, name="msb")
                    ob = opool.tile([128, HO, 27, 2], dt.int32, name="ob")
                    M = [msb[:, j, 0:NTOT] for j in range(NJ)]
                    s12 = d12 = y1 = None
                    for j in JORDER:
                        ps = [ppool.tile([128, NCOLS], dt.float32,
                                         name="ps", tag="ps")
                              for _ in range(NCH)]
                        for k in range(KT):
                            for r in range(KH):
                                lhsT = usb[:, j, k, r, m]
                                for c in range(NCH):
                                    nc.tensor.matmul(
                                        ps[c][:, :], lhsT,
                                        vt[:, k, j].rearrange(
                                            "p (h q) -> p h q", q=NP)[
                                            :, CH * c + r:CH * c + r + CH,
                                            0:27],
                                        start=(r == 0 and k == 0),
                                        stop=(r == KH - 1 and k == KT - 1))
                        # drain point j to SBUF fp16 via ACT; j=1 carries
                        # the bias (coefficient +1 in both outputs).  The
                        # final tile skips the j=0 drain: the combine
                        # reads PSUM directly.
                        if not (last and j == 0):
                            for c in range(NCH):
                                dst = msb[:, j, NCOLS * c:NCOLS * (c + 1)]
                                if j == 1:
                                    nc.scalar.activation(
                                        dst, ps[c][:, :], AF.Identity,
                                        bias=b2[:, m:m + 1], scale=1.0)
                                else:
                                    nc.scalar.activation(dst, ps[c][:, :],
                                                         AF.Copy)
                        # ---- pipelined A^T combines ----
                        if j == 2:
                            s12 = epool.tile([128, NTOT], f16, name="c0")
                            d12 = epool.tile([128, NTOT], f16, name="c1")
                            nc.vector.tensor_tensor(s12[:, :], M[1], M[2],
                                                    ALU.add)
                            nc.vector.tensor_tensor(d12[:, :], M[1], M[2],
                                                    ALU.subtract)
                        elif j == 3:
                            y1 = ypool.tile([128, NTOT], f16, name="y1")
                            nc.vector.tensor_tensor(y1[:, :], d12[:, :],
                                                    M[3], ALU.subtract)
                            nc.vector.tensor_scalar(
                                ob[:, :, :, 1],
                                y1[:, :].rearrange("p (c h q) -> p (c h) q",
                                                   c=NCH, q=27),
                                MAGIC, MAGIC, ALU.add, ALU.subtract)
                        elif j == 0:
                            if not last:
                                y0 = ypool.tile([128, NTOT], f16, name="y0")
                                nc.vector.tensor_tensor(y0[:, :], s12[:, :],
                                                        M[0], ALU.add)
                                nc.vector.tensor_scalar(
                                    ob[:, :, :, 0],
                                    y0[:, :].rearrange(
                                        "p (c h q) -> p (c h) q",
                                        c=NCH, q=27),
                                    MAGIC, MAGIC, ALU.add, ALU.subtract)
                                nc.sync.dma_start(
                                    out=out_d[n, m * 128:(m + 1) * 128],
                                    in_=ob[:, :, :, :].rearrange(
                                        "p h q f -> p h (q f)"))
                            else:
                                # fused drain+combine per chunk; chunk DMAs
                                # go out on separate queues
                                for c in range(NCH):
                                    y0 = ypool.tile([128, NCOLS], f16,
                                                    name="y0l")
                                    nc.vector.tensor_tensor(
                                        y0[:, :],
                                        s12[:, NCOLS * c:NCOLS * (c + 1)],
                                        ps[c][:, :], ALU.add)
                                    nc.vector.tensor_scalar(
                                        ob[:, CH * c:CH * (c + 1), :, 0],
                                        y0[:, :].rearrange(
                                            "p (h q) -> p h q", q=27),
                                        MAGIC, MAGIC, ALU.add, ALU.subtract)
                                    eng = (nc.sync, nc.sync,
                                           nc.gpsimd)[c]
                                    eng.dma_start(
                                        out=out_d[n, m * 128:(m + 1) * 128,
                                                  CH * c:CH * (c + 1)],
                                        in_=ob[:, CH * c:CH * (c + 1)]
                                        .rearrange("p h q f -> p h (q f)"))
                    # prep for next image on otherwise-idle slots
                    if not last_img:
                        xbn, dpn, vtn = xbs[n + 1], dps[n + 1], vts[n + 1]
                        if m == 0:
                            # DVE deals e1/o/o1; ACT deals e
                            deal(nc.scalar, xbn, dpn, 0, 0)
                            deal(nc.scalar, xbn, dpn, 1, 0)
                            for s, k in ((2, 0), (1, 0), (2, 1), (1, 1),
                                         (3, 0), (3, 1)):
                                deal(nc.vector, xbn, dpn, k, s)
                            for jj in (1, 2):
                                for k in range(KT):
                                    vtrans(dpn, vtn, k, jj)
                        else:
                            for jj in (3, 0):
                                for k in range(KT):
                                    vtrans(dpn, vtn, k, jj)

    nc.compile()
    return nc


def make_in_maps(inputs):
    x = np.ascontiguousarray(np.asarray(inputs["inputVec"], dtype=np.int8))
    w = np.asarray(inputs["weight"], dtype=np.int8)
    scales = np.asarray(inputs["scales"], dtype=np.float32)
    zp = np.asarray(inputs["zeropoints"], dtype=np.int32)
    bias = np.asarray(inputs["bias"], dtype=np.float32)
    assert x.shape == (N, CIN, H, W) and w.shape == (COUT, CIN, KH, KW)

    # host prep: fold per-channel dequant + 0.01 into transformed weights
    wq = (w.astype(np.float64) - zp[:, None, None, None]) \
        * (0.01 * scales.astype(np.float64))[:, None, None, None]
    # U[o,i,r,j] = sum_c G[j,c] wq[o,i,r,c]
    U = np.einsum("jc,oirc->oirj", GMAT, wq)
    # layout [j, cin_part, k, r, m, cout_part]
    Ur = U.reshape(MT, 128, KT, 128, KH, NJ).transpose(5, 3, 2, 4, 0, 1)
    u_h = np.ascontiguousarray(Ur, dtype=np.float16)
    # fold the x-7 per-tensor zp into bias: -7 * 0.01*scale * sum(w-zp)
    w1z = (w.astype(np.float64) - zp[:, None, None, None]).sum(axis=(1, 2, 3))
    b2 = (bias.astype(np.float64)
          - 0.07 * scales.astype(np.float64) * w1z).astype(np.float32)
    return [
        {"x": np.ascontiguousarray(x[c * NPER:(c + 1) * NPER]),
         "u": u_h, "bias2": b2}
        for c in range(NCORES)
    ]


def kernel(**inputs) -> np.ndarray:
    if "nc" not in _CACHE:
        _CACHE["nc"] = _build_program()
    nc = _CACHE["nc"]

    in_maps = make_in_maps(inputs)
    res = run_bass_kernel_spmd(nc, in_maps, list(range(NCORES)))
    out = np.concatenate([res.results[c]["out"] for c in range(NCORES)],
                         axis=0)
    return out


# revision 14
# speedup vs baseline: 1.0314x; 1.0314x over previous
"""Trainium2 Bass kernel for quantized int8 per-channel Conv2d.

Reference semantics (fp32):
  x_f = (x_int8 - 7) * 0.01                      # per-tensor dequant
  w_f = (w_int8 - zp[cout]) * scale[cout]        # per-channel dequant
  y   = round(conv2d_valid(x_f, w_f) + bias[cout])  -> int32

Algorithm: 1D Winograd F(4,3) along the HEIGHT axis, direct 3-tap
accumulation along width.  Per 4 output rows the row-conv needs 6 matmul
points instead of 12 -> PE work drops 2x vs direct conv (1.33x vs the
F(2,3) variant), and row-tiling keeps the innermost (width) axis dense,
so all transforms run in the DVE's 2x packed fp16 mode with NO strided
"deal" copies.

  y[4q:4q+4, w] = A^T [ (G w_col) * (B^T d) ],  d = x[4q:4q+6, w]
  B^T rows (exact in fp16, |V| <= 1280):
    b0=4d0-5d2+d4  b1=-4(d1+d2)+(d3+d4)  b2=4(d1-d2)+(d4-d3)
    b3=2(d3-d1)+(d4-d2)  b4=-2(d3-d1)+(d4-d2)  b5=-4(d3-d1)+(d5-d3)
  A^T = [[1,1,1,1,1,0],[0,1,-1,2,-2,0],[0,1,1,4,4,0],[0,1,-1,8,-8,1]]

H=56 rows = 14 quads exactly (input rows 4q..4q+5 need rows up to 57:
two zero pad rows).  U = G*(w-zp)*0.01*scale in fp16; host folds the
(x-7) zeropoint into the bias, which rides the j=1 drain (A^T column of
m1 is all ones).  Output rounding uses the engines' native fp32->int32
RNE conversion (verified on HW), so the final combines write int32
directly.

Engine split: PE j-major matmuls in order (1,2,3,4,5,0); ACT casts
x int8->fp16 and drains PSUM (+bias on j=1); DVE does B^T transforms
(tensor_tensor/scalar_tensor_tensor on dense fp16, 2x mode) and A^T
combines.  GPSIMD is idle: its SBUF path is slow and steals the DVE
port.  All DMA on the sync queue (HWDGE).  The last (n,m) fuses the
j=0 drain+combine (TT reads PSUM) per chunk.
Sharding: data-parallel over batch N=32 across 8 cores.
"""

import numpy as np

import concourse.bass as bass
import concourse.mybir as mybir
from concourse import bacc
from concourse.tile import TileContext
from concourse.bass_utils import run_bass_kernel_spmd

# Problem shapes (hardcoded per contract)
N, CIN, H, W = 32, 256, 56, 56
COUT, KH, KW = 256, 3, 3
HO, WO = H - KH + 1, W - KW + 1          # 54, 54
NCORES = 8
NPER = N // NCORES                        # images per core
HW = H * W                                # 3136
XPAD = HW + 64
KT = CIN // 128                           # 2 cin tiles
MT = COUT // 128                          # 2 cout tiles
NJ = 6                                    # F(4,3) winograd points
NQ = 14                                   # row quads (4 out rows each)
XR = 60                                   # padded rows in fp16 x buffer
PQ = NQ * W                               # 784 cols per V plane
NCH = 2                                   # chunks per (img, m, j)
QCH = NQ // NCH                           # 7 quads per chunk
NCOLS = QCH * WO                          # 378 matmul free dim
NTOT = NQ * WO                            # 756 combine width
MPIT = NTOT + 2                           # msb plane pitch (4B aligned)
JORDER = (1, 2, 3, 4, 5, 0)

G4 = np.array([
    [1 / 4, 0, 0],
    [-1 / 6, -1 / 6, -1 / 6],
    [-1 / 6, 1 / 6, -1 / 6],
    [1 / 24, 1 / 12, 1 / 6],
    [1 / 24, -1 / 12, 1 / 6],
    [0, 0, 1],
], dtype=np.float64)

_CACHE = {}


def _build_program():
    nc = bacc.Bacc("TRN2", target_bir_lowering=False, debug=False,
                   num_devices=NCORES)
    dt = mybir.dt
    f16 = dt.float16
    AF = mybir.ActivationFunctionType
    ALU = mybir.AluOpType

    x_d = nc.dram_tensor("x", [NPER, CIN, H, W], dt.int8, kind="ExternalInput")
    # U layout: [j, cin_part, k, c(tap), m, cout_part]  (j outer, split DMA)
    u_d = nc.dram_tensor("u", [NJ, 128, KT, KW, MT, 128], f16,
                         kind="ExternalInput")
    b2_d = nc.dram_tensor("bias2", [COUT], dt.float32, kind="ExternalInput")
    out_d = nc.dram_tensor("out", [NPER, COUT, HO, WO], dt.int32,
                           kind="ExternalOutput")

    with TileContext(nc) as tc:
        with (
            tc.tile_pool(name="const", bufs=1) as cpool,
            tc.tile_pool(name="xin", bufs=2) as xpool,
            tc.tile_pool(name="xf16", bufs=2) as fpool,
            tc.tile_pool(name="vbuf", bufs=2) as vpool,
            tc.tile_pool(name="tsc", bufs=6) as spool,
            tc.tile_pool(name="psum", bufs=6, space="PSUM") as ppool,
            tc.tile_pool(name="msb", bufs=2) as mpool,
            tc.tile_pool(name="csc", bufs=2) as epool,
            tc.tile_pool(name="outb", bufs=2) as opool,
        ):
            # ---- constants ----
            usb = cpool.tile([128, NJ, KT, KW, MT, 128], f16)
            b2 = cpool.tile([128, MT], dt.float32)

            # PE warm-up (HAM clock gate) while DMAs land
            wupw = cpool.tile([128, 128], f16)
            nc.vector.memset(wupw[:, :], 1.0)
            wupx = cpool.tile([128, 512], f16)
            nc.vector.memset(wupx[:, :], 1.0)
            wups = ppool.tile([128, 512], dt.float32, name="wups", tag="wup",
                              bufs=1)
            for _ in range(10):
                nc.tensor.matmul(wups[:, :], wupw[:, :], wupx[:, :],
                                 start=True, stop=True)

            def keepers(rhs, count):
                for _ in range(count):
                    nc.tensor.matmul(wups[:, 0:rhs.shape[-1]], wupw[:, :],
                                     rhs, start=True, stop=True)

            def xdma(n, xb):
                for k in range(KT):
                    nc.sync.dma_start(
                        out=xb[:, k, 0:HW],
                        in_=x_d[n, k * 128:(k + 1) * 128].rearrange(
                            "p h w -> p (h w)"))

            def cast(eng, xb, xf, k):
                # int8 [HW] -> fp16 rows 0..55 of the [XR, W] buffer
                dst = xf[:, k].rearrange("p r w -> p (r w)")[:, 0:HW]
                if eng is nc.scalar:
                    eng.copy(dst, xb[:, k, 0:HW])
                else:
                    eng.tensor_copy(dst, xb[:, k, 0:HW])

            def dview(xf, k, s):
                # d_s[q, w] = x[4q + s, w], dense width
                xq = xf[:, k].rearrange("p (q f) w -> p q f w", f=4)
                if s < 4:
                    return xq[:, 0:NQ, s]
                return xq[:, 1:NQ + 1, s - 4]

            ts_live = {}

            def transforms(xf, vt, part):
                # B^T for both k, j-priority order; part 0: j=1,2,3;
                # part 1: j=4,5,0 (e/f scratch carried in ts_live)
                STT = nc.vector.scalar_tensor_tensor
                TT = nc.vector.tensor_tensor
                d = [[dview(xf, k, s) for s in range(6)] for k in range(KT)]

                def sc(name, k):
                    t = spool.tile([128, NQ, W], f16, name="ts")
                    ts_live[(name, k)] = t
                    return t[:, :, :]

                def g(name, k):
                    return ts_live[(name, k)][:, :, :]

                if part == 0:
                    for k in range(KT):
                        TT(sc("p1", k), d[k][1], d[k][2], ALU.add)
                        TT(sc("p3", k), d[k][3], d[k][4], ALU.add)
                        STT(vt[:, k, 1], g("p1", k), -4.0, g("p3", k),
                            ALU.mult, ALU.add)
                    for k in range(KT):
                        TT(sc("m1", k), d[k][1], d[k][2], ALU.subtract)
                        TT(sc("m3", k), d[k][4], d[k][3], ALU.subtract)
                        STT(vt[:, k, 2], g("m1", k), 4.0, g("m3", k),
                            ALU.mult, ALU.add)
                    for k in range(KT):
                        TT(sc("e", k), d[k][3], d[k][1], ALU.subtract)
                        TT(sc("f", k), d[k][4], d[k][2], ALU.subtract)
                        STT(vt[:, k, 3], g("e", k), 2.0, g("f", k),
                            ALU.mult, ALU.add)
                else:
                    for k in range(KT):
                        STT(vt[:, k, 4], g("e", k), -2.0, g("f", k),
                            ALU.mult, ALU.add)
                    for k in range(KT):
                        TT(sc("u2", k), d[k][5], d[k][3], ALU.subtract)
                        STT(vt[:, k, 5], g("e", k), -4.0, g("u2", k),
                            ALU.mult, ALU.add)
                    for k in range(KT):
                        TT(sc("u1", k), d[k][0], d[k][2], ALU.subtract)
                        STT(vt[:, k, 0], g("u1", k), 4.0, g("f", k),
                            ALU.mult, ALU.add)

            # ---- startup DMAs: x image 0 first, then U (j=1 first) ----
            xb0 = xpool.tile([128, KT, XPAD], dt.int8, name="xb")
            xf0 = fpool.tile([128, KT, XR, W], f16, name="xf")
            vt0 = vpool.tile([128, KT, NJ, NQ, W], f16, name="vt")
            xdma(0, xb0)
            nc.sync.dma_start(out=usb[:, 1], in_=u_d[1])
            nc.sync.dma_start(out=b2[:, :],
                              in_=b2_d.rearrange("(m p) -> p m", p=128))
            for j in (2, 3, 4, 5, 0):
                nc.sync.dma_start(out=usb[:, j], in_=u_d[j])

            # ---- image 0 prologue ----
            nc.vector.memset(xf0[:, :, H:H + 2, :], 0.0)
            cast(nc.vector, xb0, xf0, 0)
            cast(nc.scalar, xb0, xf0, 1)
            transforms(xf0, vt0, 0)
            transforms(xf0, vt0, 1)
            keepers(vt0[:, 0, 1].rearrange("p q w -> p (q w)")[:, 0:512], 10)

            xbs, xfs, vts = [xb0], [xf0], [vt0]
            for n in range(NPER):
                vt = vts[n]
                last_img = n == NPER - 1
                if not last_img:
                    xbn = xpool.tile([128, KT, XPAD], dt.int8, name="xb")
                    xfn = fpool.tile([128, KT, XR, W], f16, name="xf")
                    vtn = vpool.tile([128, KT, NJ, NQ, W], f16, name="vt")
                    xdma(n + 1, xbn)
                    xbs.append(xbn)
                    xfs.append(xfn)
                    vts.append(vtn)

                for m in range(MT):
                    last = last_img and m == MT - 1
                    msb = mpool.tile([128, NJ, NQ, WO], f16, name="msb")
                    ob = opool.tile([128, H, WO], dt.int32, name="ob")
                    obq = ob[:, :, :].rearrange("p (q f) w -> p q f w", f=4)
                    M = [msb[:, j] for j in range(NJ)]
                    s12 = d12 = s34 = d34 = t0 = u8 = None
                    for j in JORDER:
                        ps = [ppool.tile([128, QCH, WO], dt.float32,
                                         name="ps", tag="ps")
                              for _ in range(NCH)]
                        for k in range(KT):
                            for c in range(KW):
                                lhsT = usb[:, j, k, c, m]
                                for ch in range(NCH):
                                    nc.tensor.matmul(
                                        ps[ch][:, :, :], lhsT,
                                        vt[:, k, j, QCH * ch:QCH * (ch + 1),
                                           c:c + WO],
                                        start=(c == 0 and k == 0),
                                        stop=(c == KW - 1 and k == KT - 1))
                        if not (last and j == 0):
                            for ch in range(NCH):
                                dst = msb[:, j, QCH * ch:QCH * (ch + 1)]
                                if j == 1:
                                    nc.scalar.activation(
                                        dst, ps[ch][:, :, :], AF.Identity,
                                        bias=b2[:, m:m + 1], scale=1.0)
                                else:
                                    nc.scalar.activation(dst,
                                                         ps[ch][:, :, :],
                                                         AF.Copy)
                        # ---- pipelined A^T combines (int32 writes: RNE) ----
                        TT = nc.vector.tensor_tensor
                        STT = nc.vector.scalar_tensor_tensor
                        if j == 2:
                            s12 = epool.tile([128, NQ, WO], f16, name="c0")
                            d12 = epool.tile([128, NQ, WO], f16, name="c1")
                            TT(s12[:, :, :], M[1], M[2], ALU.add)
                            TT(d12[:, :, :], M[1], M[2], ALU.subtract)
                        elif j == 4:
                            s34 = epool.tile([128, NQ, WO], f16, name="c2")
                            d34 = epool.tile([128, NQ, WO], f16, name="c3")
                            t0 = epool.tile([128, NQ, WO], f16, name="c4")
                            u8 = epool.tile([128, NQ, WO], f16, name="c5")
                            TT(s34[:, :, :], M[3], M[4], ALU.add)
                            TT(d34[:, :, :], M[3], M[4], ALU.subtract)
                            STT(obq[:, :, 1, :], d34[:, :, :], 2.0,
                                d12[:, :, :], ALU.mult, ALU.add)
                            STT(obq[:, :, 2, :], s34[:, :, :], 4.0,
                                s12[:, :, :], ALU.mult, ALU.add)
                            TT(t0[:, :, :], s12[:, :, :], s34[:, :, :],
                               ALU.add)
                            STT(u8[:, :, :], d34[:, :, :], 8.0,
                                d12[:, :, :], ALU.mult, ALU.add)
                        elif j == 5:
                            TT(obq[:, :, 3, :], u8[:, :, :], M[5], ALU.add)
                        elif j == 0:
                            if not last:
                                TT(obq[:, :, 0, :], t0[:, :, :], M[0],
                                   ALU.add)
                                nc.sync.dma_start(
                                    out=out_d[n, m * 128:(m + 1) * 128],
                                    in_=ob[:, 0:HO, :])
                            else:
                                # fused drain+combine per chunk; chunk DMAs
                                for ch in range(NCH):
                                    TT(obq[:, QCH * ch:QCH * (ch + 1), 0, :],
                                       t0[:, QCH * ch:QCH * (ch + 1), :],
                                       ps[ch][:, :, :], ALU.add)
                                    r0, r1 = 4 * QCH * ch, 4 * QCH * (ch + 1)
                                    r1 = min(r1, HO)
                                    nc.sync.dma_start(
                                        out=out_d[n, m * 128:(m + 1) * 128,
                                                  r0:r1],
                                        in_=ob[:, r0:r1, :])
                    # prep for next image on otherwise-idle slots
                    if not last_img:
                        xbn, xfn, vtn = xbs[n + 1], xfs[n + 1], vts[n + 1]
                        if m == 0:
                            nc.vector.memset(xfn[:, :, H:H + 2, :], 0.0)
                            cast(nc.scalar, xbn, xfn, 0)
                            cast(nc.scalar, xbn, xfn, 1)
                            transforms(xfn, vtn, 0)
                        else:
                            transforms(xfn, vtn, 1)

    nc.compile()
    return nc


def make_in_maps(inputs):
    x = np.ascontiguousarray(np.asarray(inputs["inputVec"], dtype=np.int8))
    w = np.asarray(inputs["weight"], dtype=np.int8)
    scales = np.asarray(inputs["scales"], dtype=np.float32)
    zp = np.asarray(inputs["zeropoints"], dtype=np.int32)
    bias = np.asarray(inputs["bias"], dtype=np.float32)
    assert x.shape == (N, CIN, H, W) and w.shape == (COUT, CIN, KH, KW)

    # host prep: fold per-channel dequant + 0.01 into transformed weights
    wq = (w.astype(np.float64) - zp[:, None, None, None]) \
        * (0.01 * scales.astype(np.float64))[:, None, None, None]
    # U[j,o,i,c] = sum_r G4[j,r] wq[o,i,r,c]   (transform over row taps)
    U = np.einsum("jr,oirc->joic", G4, wq)
    # layout [j, cin_part, k, c, m, cout_part]
    Ur = U.reshape(NJ, MT, 128, KT, 128, KW).transpose(0, 4, 3, 5, 1, 2)
    u_h = np.ascontiguousarray(Ur, dtype=np.float16)
    # fold the x-7 per-tensor zp into bias: -7 * 0.01*scale * sum(w-zp)
    w1z = (w.astype(np.float64) - zp[:, None, None, None]).sum(axis=(1, 2, 3))
    b2 = (bias.astype(np.float64)
          - 0.07 * scales.astype(np.float64) * w1z).astype(np.float32)
    return [
        {"x": np.ascontiguousarray(x[c * NPER:(c + 1) * NPER]),
         "u": u_h, "bias2": b2}
        for c in range(NCORES)
    ]


def kernel(**inputs) -> np.ndarray:
    if "nc" not in _CACHE:
        _CACHE["nc"] = _build_program()
    nc = _CACHE["nc"]

    in_maps = make_in_maps(inputs)
    res = run_bass_kernel_spmd(nc, in_maps, list(range(NCORES)))
    out = np.concatenate([res.results[c]["out"] for c in range(NCORES)],
                         axis=0)
    return out


# revision 16
# speedup vs baseline: 1.1715x; 1.1358x over previous
"""Trainium2 Bass kernel for quantized int8 per-channel Conv2d.

Reference semantics (fp32):
  x_f = (x_int8 - 7) * 0.01
  w_f = (w_int8 - zp[cout]) * scale[cout]
  y   = round(conv2d_valid(x_f, w_f) + bias[cout])  -> int32

Algorithm: 1D Winograd along the HEIGHT axis (width taps direct),
ALTERNATING per image between F(2,3) (4 points, cheap transforms) and
F(4,3) (6 points, 2x less PE work, heavier transforms).  The mix
balances the two bottleneck engines: PE ~108us, DVE ~95us per core.
Row-tiling keeps the innermost (width) axis dense so every transform
runs in the DVE's 2x packed fp16 mode with no strided "deal" copies.

F(2,3):  V = [r0-r2, r1+r2, r2-r1, r1-r3]  (27 row-pairs, exact)
         y0 = m0+m1+m2, y1 = m1-m2-m3
F(4,3):  14 quads of 4 rows (input rows up to 57: 2 zero pad rows)
         b0=4(d0-d2)+(d4-d2) b1=-4(d1+d2)+(d3+d4) b2=4(d1-d2)+(d4-d3)
         b3=2e+f b4=-2e+f b5=-4e+(d5-d3)   [e=d3-d1, f=d4-d2]
         A^T=[[1,1,1,1,1,0],[0,1,-1,2,-2,0],[0,1,1,4,4,0],[0,1,-1,8,-8,1]]

U = G*(w-zp)*0.01*scale in fp16; the (x-7) zeropoint folds into the
bias, which rides the j=1 drain (m1's A^T column is all ones).  Output
rounding uses the engines' native fp32->int32 RNE conversion (verified
on HW): the final combines write int32 directly.

Engines: PE j-major matmuls (j=1 first); ACT casts int8->fp16 and
drains PSUM; DVE transforms + A^T combines.  GPSIMD idle (slow SBUF
path + steals the DVE port).  All DMA via sync queue (HWDGE).  Next
image's transform groups are emitted one per combine slot to avoid
bursty DVE queues.  The last (n,m) fuses the j=0 drain+combine (TT
reads PSUM) per chunk.  Sharding: batch 32 over 8 cores.
"""

import numpy as np

import concourse.bass as bass
import concourse.mybir as mybir
from concourse import bacc
from concourse.tile import TileContext
from concourse.bass_utils import run_bass_kernel_spmd

N, CIN, H, W = 32, 256, 56, 56
COUT, KH, KW = 256, 3, 3
HO, WO = H - KH + 1, W - KW + 1          # 54, 54
NCORES = 8
NPER = N // NCORES
HW = H * W
XPAD = HW + 64
KT = CIN // 128
MT = COUT // 128
XR = 60                                   # padded rows in fp16 x buffer

NJ4 = 6                                   # F(4,3) points
NQ4 = 14                                  # row quads
QCH4 = 7                                  # quads per chunk
NCH4 = 2
J4 = (1, 2, 3, 4, 5, 0)

NJ2 = 4                                   # F(2,3) points
NQ2 = 27                                  # row pairs
QCH2 = 9                                  # pairs per chunk
NCH2 = 3
J2 = (1, 2, 3, 0)

KINDS = (2, 4, 2, 4)                      # per-image variant

G4 = np.array([
    [1 / 4, 0, 0],
    [-1 / 6, -1 / 6, -1 / 6],
    [-1 / 6, 1 / 6, -1 / 6],
    [1 / 24, 1 / 12, 1 / 6],
    [1 / 24, -1 / 12, 1 / 6],
    [0, 0, 1],
], dtype=np.float64)
G2 = np.array([
    [1, 0, 0],
    [0.5, 0.5, 0.5],
    [0.5, -0.5, 0.5],
    [0, 0, 1],
], dtype=np.float64)

_CACHE = {}


def _build_program():
    nc = bacc.Bacc("TRN2", target_bir_lowering=False, debug=False,
                   num_devices=NCORES)
    dt = mybir.dt
    f16 = dt.float16
    AF = mybir.ActivationFunctionType
    ALU = mybir.AluOpType

    x_d = nc.dram_tensor("x", [NPER, CIN, H, W], dt.int8, kind="ExternalInput")
    u4_d = nc.dram_tensor("u4", [NJ4, 128, KT, KW, MT, 128], f16,
                          kind="ExternalInput")
    u2_d = nc.dram_tensor("u2", [NJ2, 128, KT, KW, MT, 128], f16,
                          kind="ExternalInput")
    b2_d = nc.dram_tensor("bias2", [COUT], dt.float32, kind="ExternalInput")
    out_d = nc.dram_tensor("out", [NPER, COUT, HO, WO], dt.int32,
                           kind="ExternalOutput")

    with TileContext(nc) as tc:
        with (
            tc.tile_pool(name="const", bufs=1) as cpool,
            tc.tile_pool(name="xin", bufs=2) as xpool,
            tc.tile_pool(name="xf16", bufs=1) as fpool,
            tc.tile_pool(name="v4", bufs=1) as v4pool,
            tc.tile_pool(name="v2", bufs=1) as v2pool,
            tc.tile_pool(name="tsc", bufs=4) as spool,
            tc.tile_pool(name="psum", bufs=7, space="PSUM") as ppool,
            tc.tile_pool(name="m4", bufs=2) as m4pool,
            tc.tile_pool(name="m2", bufs=2) as m2pool,
            tc.tile_pool(name="csc", bufs=1) as epool,
            tc.tile_pool(name="ob4", bufs=1) as o4pool,
            tc.tile_pool(name="ob2", bufs=1) as o2pool,
        ):
            u4sb = cpool.tile([128, NJ4, KT, KW, MT, 128], f16)
            u2sb = cpool.tile([128, NJ2, KT, KW, MT, 128], f16)
            b2 = cpool.tile([128, MT], dt.float32)

            wupw = cpool.tile([128, 128], f16)
            nc.vector.memset(wupw[:, :], 1.0)
            wupx = cpool.tile([128, 512], f16)
            nc.vector.memset(wupx[:, :], 1.0)
            wups = ppool.tile([128, 512], dt.float32, name="wups", tag="wup",
                              bufs=1)
            for _ in range(10):
                nc.tensor.matmul(wups[:, :], wupw[:, :], wupx[:, :],
                                 start=True, stop=True)

            def keepers(rhs, count):
                for _ in range(count):
                    nc.tensor.matmul(wups[:, 0:rhs.shape[-1]], wupw[:, :],
                                     rhs, start=True, stop=True)

            def xdma(n, xb):
                for k in range(KT):
                    nc.sync.dma_start(
                        out=xb[:, k, 0:HW],
                        in_=x_d[n, k * 128:(k + 1) * 128].rearrange(
                            "p h w -> p (h w)"))

            def cast(eng, xb, xf, k):
                dst = xf[:, k].rearrange("p r w -> p (r w)")[:, 0:HW]
                if eng is nc.scalar:
                    eng.copy(dst, xb[:, k, 0:HW])
                else:
                    eng.tensor_copy(dst, xb[:, k, 0:HW])

            TT = nc.vector.tensor_tensor
            STT = nc.vector.scalar_tensor_tensor

            # ---- F(4,3) transforms: merged-k groups (7 emission slots) --
            ts = {}

            def d4v(xf, s):
                xq = xf[:, :, :, :].rearrange("p k (q f) w -> p k q f w",
                                              f=4)
                if s < 4:
                    return xq[:, :, 0:NQ4, s]
                return xq[:, :, 1:NQ4 + 1, s - 4]

            def sc4(name):
                t = spool.tile([128, KT, NQ4, W], f16, name="ts")
                ts[name] = t
                return t[:, :, :, :]

            def g4(name):
                return ts[name][:, :, :, :]

            def prep4(xb, xf, vt):
                d = lambda s: d4v(xf, s)

                def g_cast():
                    nc.vector.memset(xf[:, :, H:H + 2, :], 0.0)
                    cast(nc.scalar, xb, xf, 0)
                    cast(nc.scalar, xb, xf, 1)

                def g_j1():
                    TT(sc4("p1"), d(1), d(2), ALU.add)
                    TT(sc4("p3"), d(3), d(4), ALU.add)
                    STT(vt[:, 1], g4("p1"), -4.0, g4("p3"),
                        ALU.mult, ALU.add)

                def g_j2():
                    TT(sc4("m1"), d(1), d(2), ALU.subtract)
                    TT(sc4("m3"), d(4), d(3), ALU.subtract)
                    STT(vt[:, 2], g4("m1"), 4.0, g4("m3"),
                        ALU.mult, ALU.add)

                def g_j3():
                    TT(sc4("e"), d(3), d(1), ALU.subtract)
                    TT(sc4("f"), d(4), d(2), ALU.subtract)
                    STT(vt[:, 3], g4("e"), 2.0, g4("f"), ALU.mult, ALU.add)

                def g_j4():
                    STT(vt[:, 4], g4("e"), -2.0, g4("f"), ALU.mult, ALU.add)

                def g_j5():
                    TT(sc4("u2"), d(5), d(3), ALU.subtract)
                    STT(vt[:, 5], g4("e"), -4.0, g4("u2"),
                        ALU.mult, ALU.add)

                def g_j0():
                    TT(sc4("u1"), d(0), d(2), ALU.subtract)
                    STT(vt[:, 0], g4("u1"), 4.0, g4("f"), ALU.mult, ALU.add)

                return [g_cast, g_j1, g_j2, g_j3, g_j4, g_j5, g_j0]

            # ---- F(2,3) transforms: merged-k, single-TT points ----------
            def r2v(xf, s):
                xq = xf[:, :, :, :].rearrange("p k (t f) w -> p k t f w",
                                              f=2)
                if s < 2:
                    return xq[:, :, 0:NQ2, s]
                return xq[:, :, 1:NQ2 + 1, s - 2]

            def prep2(xb, xf, vt):
                d = lambda s: r2v(xf, s)

                def g_cast():
                    cast(nc.scalar, xb, xf, 0)
                    cast(nc.scalar, xb, xf, 1)

                def g_j1():
                    TT(vt[:, 1], d(1), d(2), ALU.add)

                def g_j2():
                    TT(vt[:, 2], d(2), d(1), ALU.subtract)

                def g_j3():
                    TT(vt[:, 3], d(1), d(3), ALU.subtract)

                def g_j0():
                    TT(vt[:, 0], d(0), d(2), ALU.subtract)

                return [g_cast, g_j1, g_j2, g_j3, g_j0]

            # ---- startup DMAs: x image 0 first, then U (j=1 first) ------
            xb0 = xpool.tile([128, KT, XPAD], dt.int8, name="xb")
            xf0 = fpool.tile([128, KT, XR, W], f16, name="xf")
            vt20 = v2pool.tile([128, NJ2, KT, NQ2, W], f16, name="vt2")
            xdma(0, xb0)
            nc.sync.dma_start(out=u2sb[:, 1], in_=u2_d[1])
            nc.sync.dma_start(out=b2[:, :],
                              in_=b2_d.rearrange("(m p) -> p m", p=128))
            for j in (2, 3, 0):
                nc.sync.dma_start(out=u2sb[:, j], in_=u2_d[j])
            for j in J4:
                nc.sync.dma_start(out=u4sb[:, j], in_=u4_d[j])

            # ---- image 0 (F23) prologue: split-k for latency ------------
            cast(nc.vector, xb0, xf0, 0)
            cast(nc.scalar, xb0, xf0, 1)
            for j in (1, 2, 3, 0):
                a, b, op = {1: (1, 2, ALU.add), 2: (2, 1, ALU.subtract),
                            3: (1, 3, ALU.subtract),
                            0: (0, 2, ALU.subtract)}[j]
                for k in range(KT):
                    TT(vt20[:, j, k], r2v(xf0, a)[:, k], r2v(xf0, b)[:, k],
                       op)
            keepers(vt20[:, 1, 0].rearrange("p q w -> p (q w)")[:, 0:512],
                    8)

            xfs = {0: xf0}
            vts = {0: vt20}
            preps = []

            for n in range(NPER):
                kind = KINDS[n]
                vt = vts[n]
                last_img = n == NPER - 1
                if not last_img:
                    nkind = KINDS[n + 1]
                    xbn = xpool.tile([128, KT, XPAD], dt.int8, name="xb")
                    xfn = fpool.tile([128, KT, XR, W], f16, name="xf")
                    if nkind == 4:
                        vtn = v4pool.tile([128, NJ4, KT, NQ4, W], f16,
                                          name="vt4")
                        preps = prep4(xbn, xfn, vtn)
                    else:
                        vtn = v2pool.tile([128, NJ2, KT, NQ2, W], f16,
                                          name="vt2")
                        preps = prep2(xbn, xfn, vtn)
                    xdma(n + 1, xbn)
                    xfs[n + 1] = xfn
                    vts[n + 1] = vtn
                else:
                    preps = []

                jorder = J4 if kind == 4 else J2
                nch = NCH4 if kind == 4 else NCH2
                qch = QCH4 if kind == 4 else QCH2
                nq = NQ4 if kind == 4 else NQ2
                usb = u4sb if kind == 4 else u2sb

                for m in range(MT):
                    last = last_img and m == MT - 1
                    if kind == 4:
                        msb = m4pool.tile([128, NJ4, NQ4, WO], f16,
                                          name="msb4")
                        ob = o4pool.tile([128, H, WO], dt.int32, name="ob4")
                        obq = ob[:, :, :].rearrange(
                            "p (q f) w -> p q f w", f=4)
                    else:
                        msb = m2pool.tile([128, NJ2, NQ2, WO], f16,
                                          name="msb2")
                        ob = o2pool.tile([128, NQ2, 2, WO], dt.int32,
                                         name="ob2")
                    M = [msb[:, j] for j in range(len(jorder))]
                    s12 = d12 = s34 = d34 = t0 = u8 = None
                    for j in jorder:
                        ps = [ppool.tile([128, qch, WO], dt.float32,
                                         name="ps", tag="ps")
                              for _ in range(nch)]
                        for k in range(KT):
                            for c in range(KW):
                                lhsT = usb[:, j, k, c, m]
                                for ch in range(nch):
                                    nc.tensor.matmul(
                                        ps[ch][:, :, :], lhsT,
                                        vt[:, j, k, qch * ch:qch * (ch + 1),
                                           c:c + WO],
                                        start=(c == 0 and k == 0),
                                        stop=(c == KW - 1 and k == KT - 1))
                        if not (last and j == 0):
                            for ch in range(nch):
                                dst = msb[:, j, qch * ch:qch * (ch + 1)]
                                if j == 1:
                                    nc.scalar.activation(
                                        dst, ps[ch][:, :, :], AF.Identity,
                                        bias=b2[:, m:m + 1], scale=1.0)
                                else:
                                    nc.scalar.activation(
                                        dst, ps[ch][:, :, :], AF.Copy)
                        # ---- combines (RNE int32 writes) + prep slots ---
                        if kind == 4:
                            if j == 2:
                                s12 = epool.tile([128, NQ4, WO], f16,
                                                 name="c0")
                                d12 = epool.tile([128, NQ4, WO], f16,
                                                 name="c1")
                                TT(s12[:, :, :], M[1], M[2], ALU.add)
                                TT(d12[:, :, :], M[1], M[2], ALU.subtract)
                            elif j == 4:
                                s34 = epool.tile([128, NQ4, WO], f16,
                                                 name="c2")
                                d34 = epool.tile([128, NQ4, WO], f16,
                                                 name="c3")
                                t0 = epool.tile([128, NQ4, WO], f16,
                                                name="c4")
                                u8 = epool.tile([128, NQ4, WO], f16,
                                                name="c5")
                                TT(s34[:, :, :], M[3], M[4], ALU.add)
                                TT(d34[:, :, :], M[3], M[4], ALU.subtract)
                                STT(obq[:, :, 1, :], d34[:, :, :], 2.0,
                                    d12[:, :, :], ALU.mult, ALU.add)
                                STT(obq[:, :, 2, :], s34[:, :, :], 4.0,
                                    s12[:, :, :], ALU.mult, ALU.add)
                                TT(t0[:, :, :], s12[:, :, :], s34[:, :, :],
                                   ALU.add)
                                STT(u8[:, :, :], d34[:, :, :], 8.0,
                                    d12[:, :, :], ALU.mult, ALU.add)
                            elif j == 5:
                                TT(obq[:, :, 3, :], u8[:, :, :], M[5],
                                   ALU.add)
                            elif j == 0:
                                if not last:
                                    TT(obq[:, :, 0, :], t0[:, :, :], M[0],
                                       ALU.add)
                                    nc.sync.dma_start(
                                        out=out_d[n, m * 128:(m + 1) * 128],
                                        in_=ob[:, 0:HO, :])
                                else:
                                    for ch in range(NCH4):
                                        TT(obq[:, QCH4 * ch:
                                               QCH4 * (ch + 1), 0, :],
                                           t0[:, QCH4 * ch:QCH4 * (ch + 1),
                                              :],
                                           ps[ch][:, :, :], ALU.add)
                                        r0 = 4 * QCH4 * ch
                                        r1 = min(4 * QCH4 * (ch + 1), HO)
                                        nc.sync.dma_start(
                                            out=out_d[n,
                                                      m * 128:(m + 1) * 128,
                                                      r0:r1],
                                            in_=ob[:, r0:r1, :])
                        else:
                            if j == 2:
                                s12 = epool.tile([128, NQ2, WO], f16,
                                                 name="c6")
                                d12 = epool.tile([128, NQ2, WO], f16,
                                                 name="c7")
                                TT(s12[:, :, :], M[1], M[2], ALU.add)
                                TT(d12[:, :, :], M[1], M[2], ALU.subtract)
                            elif j == 3:
                                TT(ob[:, :, 1, :], d12[:, :, :], M[3],
                                   ALU.subtract)
                            elif j == 0:
                                TT(ob[:, :, 0, :], s12[:, :, :], M[0],
                                   ALU.add)
                                nc.sync.dma_start(
                                    out=out_d[n, m * 128:(m + 1) * 128],
                                    in_=ob[:, :, :, :].rearrange(
                                        "p t f w -> p (t f) w"))
                        if preps:
                            preps.pop(0)()

    nc.compile()
    return nc


def make_in_maps(inputs):
    x = np.ascontiguousarray(np.asarray(inputs["inputVec"], dtype=np.int8))
    w = np.asarray(inputs["weight"], dtype=np.int8)
    scales = np.asarray(inputs["scales"], dtype=np.float32)
    zp = np.asarray(inputs["zeropoints"], dtype=np.int32)
    bias = np.asarray(inputs["bias"], dtype=np.float32)
    assert x.shape == (N, CIN, H, W) and w.shape == (COUT, CIN, KH, KW)

    wq = (w.astype(np.float64) - zp[:, None, None, None]) \
        * (0.01 * scales.astype(np.float64))[:, None, None, None]
    # U[j,o,i,c] = sum_r G[j,r] wq[o,i,r,c]   (transform over row taps)
    U4 = np.einsum("jr,oirc->joic", G4, wq)
    u4 = np.ascontiguousarray(
        U4.reshape(NJ4, MT, 128, KT, 128, KW).transpose(0, 4, 3, 5, 1, 2),
        dtype=np.float16)
    U2 = np.einsum("jr,oirc->joic", G2, wq)
    u2 = np.ascontiguousarray(
        U2.reshape(NJ2, MT, 128, KT, 128, KW).transpose(0, 4, 3, 5, 1, 2),
        dtype=np.float16)
    w1z = (w.astype(np.float64) - zp[:, None, None, None]).sum(axis=(1, 2, 3))
    b2 = (bias.astype(np.float64)
          - 0.07 * scales.astype(np.float64) * w1z).astype(np.float32)
    return [
        {"x": np.ascontiguousarray(x[c * NPER:(c + 1) * NPER]),
         "u4": u4, "u2": u2, "bias2": b2}
        for c in range(NCORES)
    ]


def kernel(**inputs) -> np.ndarray:
    if "nc" not in _CACHE:
        _CACHE["nc"] = _build_program()
    nc = _CACHE["nc"]

    in_maps = make_in_maps(inputs)
    res = run_bass_kernel_spmd(nc, in_maps, list(range(NCORES)))
    out = np.concatenate([res.results[c]["out"] for c in range(NCORES)],
                         axis=0)
    return out
